# revision 12
# baseline (speedup 1.0000x reference)
"""Trainium2 Bass kernel for nn_Attention_53077205844237 (GNN edge softmax).

Computation (reference):
    q   = x_j + e_ij                          # [E, 128]
    w   = tanh(concat([q, x_i], -1) @ W + b)  # [E, 8]
    out = segment_softmax(w, e_row)           # [E, 8], segments = rows

Problem structure (hardcoded): E = 131072 edges, IN = 128, F = 8,
N = 4096 nodes, and e_row = repeat(arange(4096), 32) -- every segment is a
contiguous, 32-edge block.  The segment softmax is therefore a softmax over
fixed 32-edge groups.  Since |tanh| < 1, exp() cannot overflow and the
segment-max subtraction is mathematically a no-op -- only a segment *sum*
is needed.

Sharding: edges are split contiguously across the 8 NeuronCores
(16384 edges = 512 whole segments per core), so the softmax is fully local
to each core: no collectives and no index tensors on device.

Device layout: inputs are passed feature-major (x^T, [128, E/8] per core) so
the contraction (over features) sits on the SBUF partition dim and the PE
matmul needs no on-device transposes.  The three matmuls per 512-edge chunk
accumulate W1^T@x_j^T + W1^T@e_ij^T + W2^T@x_i^T in PSUM (the q = x_j + e_ij
add is free via accumulation).  Four 512-edge chunks are packed into one PSUM
bank at column-group partition offsets 0/32/64/96 so the softmax vector ops
run with full free-dim density.  Output is stored as out^T [8, E/8] and
de-transposed on host.
"""

import sys
import types

if "/opt/trn_rl_repo" not in sys.path:
    sys.path.insert(0, "/opt/trn_rl_repo")

import numpy as np

# ---------------------------------------------------------------------------
# Optional NTFF-profile hook (used only when _run(trace=True); harmless else).
# The container's antenv package lacks axon_hooks; provide it so
# run_bass_kernel_spmd's trace path can find the profiler hook.
# ---------------------------------------------------------------------------
if "antenv.axon_hooks" not in sys.modules:
    _hooks_mod = types.ModuleType("antenv.axon_hooks")
    _hook_box = [None]
    _hooks_mod.set_axon_ntff_profile_hook = lambda h: _hook_box.__setitem__(0, h)
    _hooks_mod.get_axon_ntff_profile_hook = lambda: _hook_box[0]
    sys.modules["antenv.axon_hooks"] = _hooks_mod
    try:
        from trn_agent_boot.trn_boot import _ntff_profile_via_ctypes

        _hooks_mod.set_axon_ntff_profile_hook(
            _ntff_profile_via_ctypes("/opt/axon/libaxon_pjrt.so")
        )
    except Exception:
        pass

# Problem constants (hardcoded per the task contract).
E = 131072
IN = 128
F = 8
N_NODES = 4096
DEG = 32
N_CORES = 8
ES = E // N_CORES          # edges per core = 16384
LD = 4096                  # input DMA batch (edges): 2 MB per tensor per load
ST = 2048                  # compute batch (edges) = half of PSUM (4 banks)
CH = 512                   # matmul moving free dim / PSUM bank chunk
GROUPS = ST // CH          # chunks per compute batch = 4

_COMPILED = None           # cached (nc) bass module


def _build_bass():
    import concourse.bacc as bacc
    import concourse.tile as tile
    from concourse import mybir

    f32 = mybir.dt.float32
    f32r = mybir.dt.float32r
    AF = mybir.ActivationFunctionType

    nc = bacc.Bacc("TRN2", target_bir_lowering=False, debug=False,
                   num_devices=N_CORES)

    xjT = nc.dram_tensor("xjT", [IN, ES], f32r, kind="ExternalInput")
    eijT = nc.dram_tensor("eijT", [IN, ES], f32r, kind="ExternalInput")
    xiT = nc.dram_tensor("xiT", [IN, ES], f32r, kind="ExternalInput")
    w1 = nc.dram_tensor("W1", [IN, F], f32r, kind="ExternalInput")
    w2 = nc.dram_tensor("W2", [IN, F], f32r, kind="ExternalInput")
    bv = nc.dram_tensor("b", [F, 1], f32, kind="ExternalInput")
    outT = nc.dram_tensor("outT", [F, ES], f32, kind="ExternalOutput")

    # Load plan: middle loads are LD-edge blocks (2 MB DMAs); the first and
    # last are split into CH-edge pieces so the pipeline head fills fast and
    # the tail dependency chain is short.  Compute batches are <= ST edges
    # (half of PSUM) carved out of each load block.
    loads = []
    pos = 0
    for _ in range(GROUPS):
        loads.append((pos, CH))
        pos += CH
    while pos < ES - LD:
        loads.append((pos, LD))
        pos += LD
    while pos < ES:
        loads.append((pos, CH))
        pos += CH

    with tile.TileContext(nc) as tc:
        with (
            tc.tile_pool(name="consts", bufs=1) as consts,
            tc.tile_pool(name="ins", bufs=2) as ins_pool,
            tc.tile_pool(name="work", bufs=3) as work,
            tc.tile_pool(name="psum", bufs=2, space="PSUM") as psum_pool,
            tc.tile_pool(name="outp", bufs=3) as outp,
        ):
            w1_t = consts.tile([IN, F], f32r)
            nc.sync.dma_start(out=w1_t[:], in_=w1[:])
            w2_t = consts.tile([IN, F], f32r)
            nc.sync.dma_start(out=w2_t[:], in_=w2[:])
            bias_t = consts.tile([F, 1], f32)
            nc.sync.dma_start(out=bias_t[:], in_=bv[:])

            for li, (lpos, lsize) in enumerate(loads):
                lsl = slice(lpos, lpos + lsize)
                # Spread input loads over both HWDGE rings (SP + ACT).
                xi_eng = nc.sync if li % 2 == 0 else nc.scalar
                xj_t = ins_pool.tile([IN, lsize], f32r, tag="xj")
                nc.sync.dma_start(out=xj_t[:], in_=xjT[:, lsl])
                eij_t = ins_pool.tile([IN, lsize], f32r, tag="eij")
                nc.scalar.dma_start(out=eij_t[:], in_=eijT[:, lsl])
                xi_t = ins_pool.tile([IN, lsize], f32r, tag="xi")
                xi_eng.dma_start(out=xi_t[:], in_=xiT[:, lsl])

                for bpos in range(0, lsize, ST):
                    size = min(ST, lsize - bpos)
                    ngrp = size // CH
                    nseg = size // DEG
                    osl = slice(lpos + bpos, lpos + bpos + size)

                    # One 512-edge chunk per PSUM bank; partitions 0..7 = f.
                    ps_full = psum_pool.tile([F, ST], f32, tag="ps")
                    ps = ps_full[:, 0:size]
                    for g in range(ngrp):
                        csl = slice(bpos + g * CH, bpos + (g + 1) * CH)
                        po = ps[:, g * CH:(g + 1) * CH]
                        nc.tensor.matmul(po, w1_t[:], xj_t[:, csl],
                                         start=True, stop=False)
                        nc.tensor.matmul(po, w1_t[:], eij_t[:, csl],
                                         start=False, stop=False)
                        nc.tensor.matmul(po, w2_t[:], xi_t[:, csl],
                                         start=False, stop=True)

                    # ew = exp(tanh(psum + b)); |tanh| < 1, no max needed.
                    wt = work.tile([F, size], f32, tag="w")
                    nc.scalar.activation(out=wt[:], in_=ps[:], func=AF.Tanh,
                                         bias=bias_t[:, 0:1])
                    ew = work.tile([F, size], f32, tag="ew")
                    nc.scalar.activation(out=ew[:], in_=wt[:], func=AF.Exp)

                    # Segment sums over each 32-edge block, then reciprocal.
                    denom = work.tile([F, nseg], f32, tag="denom")
                    nc.vector.reduce_sum(
                        out=denom[:],
                        in_=ew[:].rearrange("p (n d) -> p n d", d=DEG),
                        axis=mybir.AxisListType.X,
                    )
                    recip = work.tile([F, nseg], f32, tag="recip")
                    nc.vector.reciprocal(out=recip[:], in_=denom[:])

                    ot = outp.tile([F, size], f32, tag="o")
                    nc.vector.tensor_mul(
                        out=ot[:].rearrange("p (n d) -> p n d", d=DEG),
                        in0=ew[:].rearrange("p (n d) -> p n d", d=DEG),
                        in1=recip[:].unsqueeze(-1).broadcast_to(
                            [F, nseg, DEG]),
                    )
                    nc.sync.dma_start(out=outT[:, osl], in_=ot[:])

    nc.compile()
    return nc


def _get_compiled():
    global _COMPILED
    if _COMPILED is None:
        _COMPILED = _build_bass()
    return _COMPILED


def _run_device(x_i, x_j, e_ij, W, b, trace=False, tmpdir=None,
                trace_cores=None):
    from concourse.bass_utils import run_bass_kernel_spmd

    nc = _get_compiled()

    W = np.ascontiguousarray(np.asarray(W, dtype=np.float32))
    b = np.asarray(b, dtype=np.float32).reshape(F, 1)
    W1 = np.ascontiguousarray(W[:IN])
    W2 = np.ascontiguousarray(W[IN:])

    in_maps = []
    for c in range(N_CORES):
        sl = slice(c * ES, (c + 1) * ES)
        in_maps.append({
            "xjT": np.ascontiguousarray(np.asarray(x_j[sl]).T),
            "eijT": np.ascontiguousarray(np.asarray(e_ij[sl]).T),
            "xiT": np.ascontiguousarray(np.asarray(x_i[sl]).T),
            "W1": W1,
            "W2": W2,
            "b": b,
        })

    kwargs = {}
    if trace:
        kwargs.update(trace=True,
                      trace_cores=(trace_cores if trace_cores is not None
                                   else list(range(N_CORES))),
                      tmpdir=tmpdir)
    res = run_bass_kernel_spmd(nc, in_maps, core_ids=list(range(N_CORES)),
                               **kwargs)

    out = np.empty((E, F), dtype=np.float32)
    for c in range(N_CORES):
        out[c * ES:(c + 1) * ES] = np.asarray(res.results[c]["outT"]).T
    return out, res


def _numpy_fallback(x_i, x_j, e_ij, adj, e_row, W, b):
    """Correct for arbitrary e_row (matches the reference semantics)."""
    x_i = np.asarray(x_i, np.float32)
    x_j = np.asarray(x_j, np.float32)
    e_ij = np.asarray(e_ij, np.float32)
    W = np.asarray(W, np.float32)
    b = np.asarray(b, np.float32)
    e_row = np.asarray(e_row).astype(np.int64)
    n = np.asarray(adj).shape[0]
    q = x_j + e_ij
    z = q @ W[:q.shape[1]] + x_i @ W[q.shape[1]:] + b
    w = np.tanh(z)
    m = np.full((n, w.shape[1]), -9e15, np.float32)
    np.maximum.at(m, e_row, w)
    ew = np.exp(w - m[e_row])
    denom = np.zeros((n, w.shape[1]), np.float32)
    np.add.at(denom, e_row, ew)
    return (ew / denom[e_row]).astype(np.float32)


def _is_fast_path(x_i, x_j, e_ij, adj, e_row, W, b):
    try:
        if np.asarray(x_i).shape != (E, IN):
            return False
        if np.asarray(x_j).shape != (E, IN):
            return False
        if np.asarray(e_ij).shape != (E, IN):
            return False
        if np.asarray(W).shape != (2 * IN, F):
            return False
        if np.asarray(b).reshape(-1).shape != (F,):
            return False
        if np.asarray(adj).shape[0] != N_NODES:
            return False
        er = np.asarray(e_row).reshape(-1)
        if er.shape != (E,):
            return False
        expected = np.repeat(np.arange(N_NODES, dtype=np.int64), DEG)
        return bool(np.array_equal(er.astype(np.int64), expected))
    except Exception:
        return False


def kernel(x_i, x_j, e_ij, adj, e_row, e_col, W, b, **_unused):
    if not _is_fast_path(x_i, x_j, e_ij, adj, e_row, W, b):
        return _numpy_fallback(x_i, x_j, e_ij, adj, e_row, W, b)
    out, _ = _run_device(x_i, x_j, e_ij, W, b)
    return out


# revision 14
# speedup vs baseline: 1.0774x; 1.0774x over previous
"""Trainium2 Bass kernel for nn_Attention_53077205844237 (GNN edge softmax).

Computation (reference):
    q   = x_j + e_ij                          # [E, 128]
    w   = tanh(concat([q, x_i], -1) @ W + b)  # [E, 8]
    out = segment_softmax(w, e_row)           # [E, 8], segments = rows

Problem structure (hardcoded): E = 131072 edges, IN = 128, F = 8,
N = 4096 nodes, and e_row = repeat(arange(4096), 32) -- every segment is a
contiguous, 32-edge block.  The segment softmax is therefore a softmax over
fixed 32-edge groups.  Since |tanh| < 1, exp() cannot overflow and the
segment-max subtraction is mathematically a no-op -- only a segment *sum*
is needed.

Sharding: edges are split contiguously across the 8 NeuronCores
(16384 edges = 512 whole segments per core), so the softmax is fully local
to each core: no collectives and no index tensors on device.

Device layout: inputs are passed feature-major (x^T, [128, E/8] per core) so
the contraction (over features) sits on the SBUF partition dim and the PE
matmul needs no on-device transposes.  The three matmuls per 512-edge chunk
accumulate W1^T@x_j^T + W1^T@e_ij^T + W2^T@x_i^T in PSUM (the q = x_j + e_ij
add is free via accumulation).  Four 512-edge chunks are packed into one PSUM
bank at column-group partition offsets 0/32/64/96 so the softmax vector ops
run with full free-dim density.  Output is stored as out^T [8, E/8] and
de-transposed on host.
"""

import sys
import types

if "/opt/trn_rl_repo" not in sys.path:
    sys.path.insert(0, "/opt/trn_rl_repo")

import numpy as np

# ---------------------------------------------------------------------------
# Optional NTFF-profile hook (used only when _run(trace=True); harmless else).
# The container's antenv package lacks axon_hooks; provide it so
# run_bass_kernel_spmd's trace path can find the profiler hook.
# ---------------------------------------------------------------------------
if "antenv.axon_hooks" not in sys.modules:
    _hooks_mod = types.ModuleType("antenv.axon_hooks")
    _hook_box = [None]
    _hooks_mod.set_axon_ntff_profile_hook = lambda h: _hook_box.__setitem__(0, h)
    _hooks_mod.get_axon_ntff_profile_hook = lambda: _hook_box[0]
    sys.modules["antenv.axon_hooks"] = _hooks_mod
    try:
        from trn_agent_boot.trn_boot import _ntff_profile_via_ctypes

        _hooks_mod.set_axon_ntff_profile_hook(
            _ntff_profile_via_ctypes("/opt/axon/libaxon_pjrt.so")
        )
    except Exception:
        pass

# Problem constants (hardcoded per the task contract).
E = 131072
IN = 128
F = 8
N_NODES = 4096
DEG = 32
N_CORES = 8
ES = E // N_CORES          # edges per core = 16384
LD = 2048                  # input DMA batch (edges): 1 MB per tensor per load
ST = 2048                  # compute batch (edges) = half of PSUM (4 banks)
CH = 512                   # matmul moving free dim / PSUM bank chunk
GROUPS = ST // CH          # chunks per compute batch = 4

_COMPILED = None           # cached (nc) bass module


def _build_bass():
    import concourse.bacc as bacc
    import concourse.tile as tile
    from concourse import mybir

    f32 = mybir.dt.float32
    f32r = mybir.dt.float32r
    AF = mybir.ActivationFunctionType

    nc = bacc.Bacc("TRN2", target_bir_lowering=False, debug=False,
                   num_devices=N_CORES)

    xjT = nc.dram_tensor("xjT", [IN, ES], f32r, kind="ExternalInput")
    eijT = nc.dram_tensor("eijT", [IN, ES], f32r, kind="ExternalInput")
    xiT = nc.dram_tensor("xiT", [IN, ES], f32r, kind="ExternalInput")
    w1 = nc.dram_tensor("W1", [IN, F], f32r, kind="ExternalInput")
    w2 = nc.dram_tensor("W2", [IN, F], f32r, kind="ExternalInput")
    bv = nc.dram_tensor("b", [F, 1], f32, kind="ExternalInput")
    outT = nc.dram_tensor("outT", [F, ES], f32, kind="ExternalOutput")

    # Load plan: middle loads are LD-edge blocks (2 MB DMAs); the first and
    # last are split into CH-edge pieces so the pipeline head fills fast and
    # the tail dependency chain is short.  Compute batches are <= ST edges
    # (half of PSUM) carved out of each load block.
    loads = []
    pos = 0
    for _ in range(GROUPS):
        loads.append((pos, CH))
        pos += CH
    while pos < ES - LD:
        loads.append((pos, LD))
        pos += LD
    while pos < ES:
        loads.append((pos, CH))
        pos += CH

    with tile.TileContext(nc) as tc:
        with (
            tc.tile_pool(name="consts", bufs=1) as consts,
            tc.tile_pool(name="ins", bufs=4) as ins_pool,
            tc.tile_pool(name="work", bufs=3) as work,
            tc.tile_pool(name="psum", bufs=2, space="PSUM") as psum_pool,
            tc.tile_pool(name="outp", bufs=3) as outp,
        ):
            w1_t = consts.tile([IN, F], f32r)
            nc.sync.dma_start(out=w1_t[:], in_=w1[:])
            w2_t = consts.tile([IN, F], f32r)
            nc.sync.dma_start(out=w2_t[:], in_=w2[:])
            bias_t = consts.tile([F, 1], f32)
            nc.sync.dma_start(out=bias_t[:], in_=bv[:])

            for li, (lpos, lsize) in enumerate(loads):
                lsl = slice(lpos, lpos + lsize)
                # Spread input loads over both HWDGE rings (SP + ACT).
                xi_eng = nc.sync if li % 2 == 0 else nc.scalar
                xj_t = ins_pool.tile([IN, lsize], f32r, tag="xj")
                nc.sync.dma_start(out=xj_t[:], in_=xjT[:, lsl])
                eij_t = ins_pool.tile([IN, lsize], f32r, tag="eij")
                nc.scalar.dma_start(out=eij_t[:], in_=eijT[:, lsl])
                xi_t = ins_pool.tile([IN, lsize], f32r, tag="xi")
                xi_eng.dma_start(out=xi_t[:], in_=xiT[:, lsl])

                for bpos in range(0, lsize, ST):
                    size = min(ST, lsize - bpos)
                    ngrp = size // CH
                    nseg = size // DEG
                    osl = slice(lpos + bpos, lpos + bpos + size)

                    # One 512-edge chunk per PSUM bank; partitions 0..7 = f.
                    ps_full = psum_pool.tile([F, ST], f32, tag="ps")
                    ps = ps_full[:, 0:size]
                    for g in range(ngrp):
                        csl = slice(bpos + g * CH, bpos + (g + 1) * CH)
                        po = ps[:, g * CH:(g + 1) * CH]
                        nc.tensor.matmul(po, w1_t[:], xj_t[:, csl],
                                         start=True, stop=False)
                        nc.tensor.matmul(po, w1_t[:], eij_t[:, csl],
                                         start=False, stop=False)
                        nc.tensor.matmul(po, w2_t[:], xi_t[:, csl],
                                         start=False, stop=True)

                    # ew = exp(tanh(psum + b)); |tanh| < 1, no max needed.
                    wt = work.tile([F, size], f32, tag="w")
                    nc.scalar.activation(out=wt[:], in_=ps[:], func=AF.Tanh,
                                         bias=bias_t[:, 0:1])
                    ew = work.tile([F, size], f32, tag="ew")
                    nc.scalar.activation(out=ew[:], in_=wt[:], func=AF.Exp)

                    # Segment sums over each 32-edge block, then reciprocal.
                    denom = work.tile([F, nseg], f32, tag="denom")
                    nc.vector.reduce_sum(
                        out=denom[:],
                        in_=ew[:].rearrange("p (n d) -> p n d", d=DEG),
                        axis=mybir.AxisListType.X,
                    )
                    recip = work.tile([F, nseg], f32, tag="recip")
                    nc.vector.reciprocal(out=recip[:], in_=denom[:])

                    ot = outp.tile([F, size], f32, tag="o")
                    nc.vector.tensor_mul(
                        out=ot[:].rearrange("p (n d) -> p n d", d=DEG),
                        in0=ew[:].rearrange("p (n d) -> p n d", d=DEG),
                        in1=recip[:].unsqueeze(-1).broadcast_to(
                            [F, nseg, DEG]),
                    )
                    nc.sync.dma_start(out=outT[:, osl], in_=ot[:])

    nc.compile()
    return nc


def _get_compiled():
    global _COMPILED
    if _COMPILED is None:
        _COMPILED = _build_bass()
    return _COMPILED


def _run_device(x_i, x_j, e_ij, W, b, trace=False, tmpdir=None,
                trace_cores=None):
    from concourse.bass_utils import run_bass_kernel_spmd

    nc = _get_compiled()

    W = np.ascontiguousarray(np.asarray(W, dtype=np.float32))
    b = np.asarray(b, dtype=np.float32).reshape(F, 1)
    W1 = np.ascontiguousarray(W[:IN])
    W2 = np.ascontiguousarray(W[IN:])

    in_maps = []
    for c in range(N_CORES):
        sl = slice(c * ES, (c + 1) * ES)
        in_maps.append({
            "xjT": np.ascontiguousarray(np.asarray(x_j[sl]).T),
            "eijT": np.ascontiguousarray(np.asarray(e_ij[sl]).T),
            "xiT": np.ascontiguousarray(np.asarray(x_i[sl]).T),
            "W1": W1,
            "W2": W2,
            "b": b,
        })

    kwargs = {}
    if trace:
        kwargs.update(trace=True,
                      trace_cores=(trace_cores if trace_cores is not None
                                   else list(range(N_CORES))),
                      tmpdir=tmpdir)
    res = run_bass_kernel_spmd(nc, in_maps, core_ids=list(range(N_CORES)),
                               **kwargs)

    out = np.empty((E, F), dtype=np.float32)
    for c in range(N_CORES):
        out[c * ES:(c + 1) * ES] = np.asarray(res.results[c]["outT"]).T
    return out, res


def _numpy_fallback(x_i, x_j, e_ij, adj, e_row, W, b):
    """Correct for arbitrary e_row (matches the reference semantics)."""
    x_i = np.asarray(x_i, np.float32)
    x_j = np.asarray(x_j, np.float32)
    e_ij = np.asarray(e_ij, np.float32)
    W = np.asarray(W, np.float32)
    b = np.asarray(b, np.float32)
    e_row = np.asarray(e_row).astype(np.int64)
    n = np.asarray(adj).shape[0]
    q = x_j + e_ij
    z = q @ W[:q.shape[1]] + x_i @ W[q.shape[1]:] + b
    w = np.tanh(z)
    m = np.full((n, w.shape[1]), -9e15, np.float32)
    np.maximum.at(m, e_row, w)
    ew = np.exp(w - m[e_row])
    denom = np.zeros((n, w.shape[1]), np.float32)
    np.add.at(denom, e_row, ew)
    return (ew / denom[e_row]).astype(np.float32)


def _is_fast_path(x_i, x_j, e_ij, adj, e_row, W, b):
    try:
        if np.asarray(x_i).shape != (E, IN):
            return False
        if np.asarray(x_j).shape != (E, IN):
            return False
        if np.asarray(e_ij).shape != (E, IN):
            return False
        if np.asarray(W).shape != (2 * IN, F):
            return False
        if np.asarray(b).reshape(-1).shape != (F,):
            return False
        if np.asarray(adj).shape[0] != N_NODES:
            return False
        er = np.asarray(e_row).reshape(-1)
        if er.shape != (E,):
            return False
        expected = np.repeat(np.arange(N_NODES, dtype=np.int64), DEG)
        return bool(np.array_equal(er.astype(np.int64), expected))
    except Exception:
        return False


def kernel(x_i, x_j, e_ij, adj, e_row, e_col, W, b, **_unused):
    if not _is_fast_path(x_i, x_j, e_ij, adj, e_row, W, b):
        return _numpy_fallback(x_i, x_j, e_ij, adj, e_row, W, b)
    out, _ = _run_device(x_i, x_j, e_ij, W, b)
    return out
